# revision 13
# baseline (speedup 1.0000x reference)
"""Trainium2 Bass kernel for nn_CustomAttention (dense_transformer).

Data-parallel over batch: 8 images -> 8 NeuronCores. Per core, one
128x128x256 image runs through:
  qkv = x @ W_qkv               (PE, fp32r, channel-major output)
  qkv = DWConv3x3(qkv)          (PE, per-channel diag-weight matmuls)
  q,k normalized per spatial position; channel attention gram k^T q
  softmax over axis -2; out = v @ attn; y = out @ W_out

Layouts: "channel-major" tiles are [C partitions, spatial free]; the
gram contraction runs in natural [spatial, C] layout via PE transposes.
"""
import sys

sys.path.insert(0, "/opt/trn_rl_repo")

import numpy as np
import concourse.bass as bass
import concourse.bacc as bacc
import concourse.mybir as mybir
from concourse import masks
from concourse.tile import TileContext
from concourse.tile_rust import add_dep_helper
from concourse.bass_utils import run_bass_kernel_spmd

F32 = mybir.dt.float32
F32R = mybir.dt.float32r
AL = mybir.AluOpType
ACTF = mybir.ActivationFunctionType

B, H, W, C = 8, 128, 128, 256
HEADS, DH = 16, 16
S = H * W                 # 16384 spatial positions
C3 = 3 * C                # 768 qkv channels
NCH = C3 // 128           # 6 channel chunks (0,1:q  2,3:k  4,5:v)
TH = 8                    # interior rows per strip
NSTRIP = H // TH          # 16
PADW = W + 2              # 130
NWIN = S // 512           # 32 (stage B windows)

_cache = {}
_last_in_maps = None


def build_nc():
    nc = bacc.Bacc("TRN2", target_bir_lowering=False, debug=False)

    x_in = nc.dram_tensor("x", [S, C], F32, kind="ExternalInput")
    wqkv_in = nc.dram_tensor("w_qkv", [C, C3], F32, kind="ExternalInput")
    wdw_in = nc.dram_tensor("w_dw", [9, C3], F32, kind="ExternalInput")
    wout_in = nc.dram_tensor("w_out", [C, C], F32, kind="ExternalInput")
    temp_in = nc.dram_tensor("temperature", [HEADS], F32, kind="ExternalInput")
    mq_in = nc.dram_tensor("mean_q", [S], F32, kind="ExternalInput")
    vq_in = nc.dram_tensor("var_q", [S], F32, kind="ExternalInput")
    mk_in = nc.dram_tensor("mean_k", [S], F32, kind="ExternalInput")
    vk_in = nc.dram_tensor("var_k", [S], F32, kind="ExternalInput")
    y_out = nc.dram_tensor("out", [S, C], F32, kind="ExternalOutput")

    with TileContext(nc) as tc:
        with (
            tc.tile_pool(name="dram", bufs=1, space="DRAM") as dp,
            tc.tile_pool(name="const", bufs=1) as cp,
            tc.tile_pool(name="xnat", bufs=3) as xnp,
            tc.tile_pool(name="strip", bufs=2) as sp,
            tc.tile_pool(name="qkt", bufs=1) as qkp,
            tc.tile_pool(name="qnat", bufs=3) as qnp,
            tc.tile_pool(name="bwin", bufs=2) as bp,
            tc.tile_pool(name="ps512", bufs=3, space="PSUM") as ps512,
            tc.tile_pool(name="ps128", bufs=2, space="PSUM") as ps128,
            tc.tile_pool(name="psattn", bufs=1, space="PSUM") as psat,
        ):
            vT_spill = [[dp.tile([128, 1024], F32, name=f"vsp{ch}_{st}")
                         for st in range(NSTRIP)] for ch in range(2)]
            gram_dram = [dp.tile([128, C], F32, name=f"gramd{m}")
                         for m in range(2)]
            attn_dram = [dp.tile([128, DH], F32, name=f"attnd{m}")
                         for m in range(2)]
            _cache["dbg"] = {
                "vT_spill": [[t.tensor.name for t in row] for row in vT_spill],
                "gram_dram": [t.tensor.name for t in gram_dram],
                "attn_dram": [t.tensor.name for t in attn_dram]}
            # ---------------- constants ----------------
            eye = cp.tile([128, 128], F32, tag="eye")
            masks.make_identity(nc, eye[:])
            eye_r = cp.tile([128, 128], F32R, tag="eye_r")
            nc.vector.tensor_copy(eye_r[:], eye[:])

            zeros = cp.tile([128, PADW * 2], F32, tag="zeros")
            nc.vector.memset(zeros[:], 0.0)

            # W_qkv as two K-chunks [128, 768] (lhsT slices [128,128])
            wq = []
            for k in range(2):
                t = cp.tile([128, C3], F32R, tag=f"wq{k}")
                nc.sync.dma_start(t[:], wqkv_in[k * 128:(k + 1) * 128, :].bitcast(F32R))
                wq.append(t)
            # W_out as two K-chunks [128, 256]
            wo = []
            for k in range(2):
                t = cp.tile([128, C], F32R, tag=f"wo{k}")
                nc.sync.dma_start(t[:], wout_in[k * 128:(k + 1) * 128, :].bitcast(F32R))
                wo.append(t)

            # depthwise weights per chunk [128, 9] and diag lhsT tiles
            diags = []
            for m in range(NCH):
                wt = cp.tile([128, 9], F32, tag=f"wdw{m}")
                nc.sync.dma_start(
                    wt[:],
                    wdw_in[:].rearrange("t c -> c t")[m * 128:(m + 1) * 128, :])
                row = []
                for t in range(9):
                    d = cp.tile([128, 128], F32R, tag=f"diag{m}_{t}")
                    nc.vector.tensor_scalar(d[:], eye[:], wt[:, t:t + 1], None,
                                            op0=AL.mult)
                    row.append(d)
                diags.append(row)

            # temperature -> [128,1], partition (h,j) = temp[j]
            ttile = cp.tile([128, 1], F32, tag="ttile")
            for h0 in range(8):
                nc.sync.dma_start(ttile[h0 * 16:(h0 + 1) * 16, :],
                                  temp_in[:].rearrange("(t o) -> t o", o=1))

            # mean / rsqrt(var) tables: [128 (s%128), 128 (s//128)]
            def load_stats(mean_dram, var_dram):
                mt = cp.tile([128, 128], F32, tag=f"m_{mean_dram.name}")
                nc.sync.dma_start(mt[:], mean_dram[:].rearrange("(a b) -> b a", b=128))
                vt = cp.tile([128, 128], F32, tag=f"v_{var_dram.name}")
                nc.sync.dma_start(vt[:], var_dram[:].rearrange("(a b) -> b a", b=128))
                st = cp.tile([128, 128], F32, tag=f"s_{var_dram.name}")
                nc.scalar.sqrt(st[:], vt[:])
                rt = cp.tile([128, 128], F32, tag=f"r_{var_dram.name}")
                nc.vector.reciprocal(rt[:], st[:])
                return mt, rt

            mq_t, rq_t = load_stats(mq_in, vq_in)
            mk_t, rk_t = load_stats(mk_in, vk_in)

            # gram accumulators: gram'[(h,j) part, (h,i) free], 2 M-chunks
            attn_ps = [psat.tile([128, C], F32, tag=f"attn{m}", name=f"attn{m}")
                       for m in range(2)]

            spill_insts = {}
            # ---------------- stage A: strips ----------------
            for s in range(NSTRIP):
                lo = 1 if s == 0 else 0          # first computable pad row
                hi = 9 if s == NSTRIP - 1 else 10

                # x rows (image rows 8s+pr-1), transposed to channel-major
                xT = [sp.tile([128, 10, 128], F32R, tag=f"xT{k}", name=f"xT{k}_{s}")
                      for k in range(2)]
                for pr in range(lo, hi):
                    ir = TH * s + pr - 1
                    xn = xnp.tile([128, C], F32, tag="xn")
                    nc.sync.dma_start(xn[:], x_in[ir * 128:(ir + 1) * 128, :])
                    for k in range(2):
                        tp = ps128.tile([128, 128], F32, tag="t128")
                        nc.tensor.transpose(tp[:], xn[:, k * 128:(k + 1) * 128], eye[:])
                        nc.scalar.copy(xT[k][:, pr, :], tp[:])

                # qkv0^T in padded layout [128, 10, 130]
                pads = [sp.tile([128, 10, PADW], F32R, tag=f"pad{m}", name=f"pad{m}_{s}")
                        for m in range(NCH)]
                for m in range(NCH):
                    # zero pad columns (w=-1 and w=128)
                    nc.vector.tensor_copy(
                        pads[m][:, :, 0:1].rearrange("p a b -> p (a b)"),
                        zeros[:, 0:10])
                    nc.vector.tensor_copy(
                        pads[m][:, :, 129:130].rearrange("p a b -> p (a b)"),
                        zeros[:, 0:10])
                    if s == 0:
                        nc.vector.tensor_copy(pads[m][:, 0, :], zeros[:, 0:PADW])
                    if s == NSTRIP - 1:
                        nc.vector.tensor_copy(pads[m][:, 9, :], zeros[:, 0:PADW])
                    r = lo
                    while r < hi:
                        n = min(4, hi - r)
                        mm = ps512.tile([128, 4, 128], F32, tag="w512")
                        for k in range(2):
                            nc.tensor.matmul(
                                mm[:, 0:n, :],
                                wq[k][:, m * 128:(m + 1) * 128],
                                xT[k][:, r:r + n, :],
                                start=(k == 0), stop=(k == 1))
                        nc.scalar.copy(pads[m][:, r:r + n, 1:129], mm[:, 0:n, :])
                        r += n

                # depthwise conv: out image rows 8s..8s+8 (pad rows 1..9)
                qkT = [qkp.tile([128, TH, 128], F32R, tag=f"qkT{m}", name=f"qkT{m}_{s}")
                       for m in range(4)]
                vhat = [qkp.tile([128, TH, 128], F32, tag=f"vh{m}", name=f"vh{m}_{s}")
                        for m in range(2)]
                for m in range(NCH):
                    for wi in range(2):
                        h0 = wi * 4
                        cv = ps512.tile([128, 4, 128], F32, tag="w512")
                        for t in range(9):
                            ky, kx = t // 3, t % 3
                            nc.tensor.matmul(
                                cv[:], diags[m][t][:],
                                pads[m][:, h0 + ky:h0 + ky + 4, kx:kx + 128],
                                start=(t == 0), stop=(t == 8))
                        if m < 4:
                            nc.scalar.copy(qkT[m][:, h0:h0 + 4, :], cv[:])
                        else:
                            nc.scalar.copy(vhat[m - 4][:, h0:h0 + 4, :], cv[:])

                # spill v̂^T
                for ch in range(2):
                    spill_insts[(ch, s)] = nc.sync.dma_start(
                        vT_spill[ch][s][:],
                        vhat[ch][:].rearrange("p a b -> p (a b)"))

                # q̂,k̂ back to natural layout + normalize, then gram
                for r in range(TH):
                    sc = TH * s + r
                    qn = qnp.tile([128, C], F32R, tag="qn")
                    kn = qnp.tile([128, C], F32R, tag="kn")
                    for m in range(2):
                        tq = ps128.tile([128, 128], F32, tag="t128")
                        nc.tensor.transpose(tq[:].bitcast(F32R),
                                            qkT[m][:, r, :], eye_r[:])
                        nc.vector.tensor_scalar(
                            qn[:, m * 128:(m + 1) * 128], tq[:],
                            mq_t[:, sc:sc + 1], rq_t[:, sc:sc + 1],
                            op0=AL.subtract, op1=AL.mult)
                        tk = ps128.tile([128, 128], F32, tag="t128")
                        nc.tensor.transpose(tk[:].bitcast(F32R),
                                            qkT[2 + m][:, r, :], eye_r[:])
                        nc.vector.tensor_scalar(
                            kn[:, m * 128:(m + 1) * 128], tk[:],
                            mk_t[:, sc:sc + 1], rk_t[:, sc:sc + 1],
                            op0=AL.subtract, op1=AL.mult)
                    first = (s == 0 and r == 0)
                    last = (s == NSTRIP - 1 and r == TH - 1)
                    for m in range(2):
                        nc.tensor.matmul(
                            attn_ps[m][:], qn[:, m * 128:(m + 1) * 128], kn[:],
                            start=first, stop=last)

            # ---------------- softmax interlude ----------------
            # attn_ps[m][(h,j) local, (h,i) global] ; want softmax over i
            asm = []
            attn_w = {}
            for m in range(2):
                # PSUM gram -> SBUF -> DRAM, then affine diag-block gather
                gsb = cp.tile([128, C], F32, tag=f"gsb{m}", name=f"gsb{m}")
                nc.vector.tensor_copy(gsb[:], attn_ps[m][:])
                gw = nc.sync.dma_start(gram_dram[m][:], gsb[:])
                pk = cp.tile([128, DH], F32, tag=f"pk{m}")
                gather = bass.AP(gram_dram[m].tensor, m * 128,
                                 [[16 * C + 16, 8], [C, 16], [1, 16]])
                pg = nc.sync.dma_start(pk[:], gather)
                add_dep_helper(pg.ins, gw.ins, reason="gram spill RAW")
                # temperature multiplies along j (= partition here)
                nc.vector.tensor_scalar(pk[:], pk[:], ttile[:], None, op0=AL.mult)
                mx = cp.tile([128, 1], F32, tag=f"mx{m}")
                nc.vector.tensor_reduce(mx[:], pk[:], axis=mybir.AxisListType.X,
                                        op=AL.max)
                nmx = cp.tile([128, 1], F32, tag=f"nmx{m}")
                nc.vector.tensor_scalar(nmx[:], mx[:], -1.0, None, op0=AL.mult)
                ex = cp.tile([128, DH], F32, tag=f"ex{m}")
                nc.scalar.activation(ex[:], pk[:], ACTF.Exp, bias=nmx[:], scale=1.0)
                sm = cp.tile([128, 1], F32, tag=f"sm{m}")
                nc.vector.tensor_reduce(sm[:], ex[:], axis=mybir.AxisListType.X,
                                        op=AL.add)
                rs = cp.tile([128, 1], F32, tag=f"rs{m}")
                nc.vector.reciprocal(rs[:], sm[:])
                sfm = cp.tile([128, DH], F32, tag=f"sfm{m}")
                nc.vector.tensor_scalar(sfm[:], ex[:], rs[:], None, op0=AL.mult)
                attn_w[m] = nc.sync.dma_start(attn_dram[m][:], sfm[:])
                asm.append(sfm)

            # block-diagonal apply matrices A[(h,i),(h,j)] = attn[i,j]

            amat = []
            for m in range(2):
                A = cp.tile([128, 128], F32R, tag=f"A{m}")
                nc.vector.tensor_copy(A[:], zeros[:, 0:128])
                for h0 in range(8):
                    ai = nc.sync.dma_start(
                        A[h0 * 16:(h0 + 1) * 16, h0 * 16:(h0 + 1) * 16],
                        attn_dram[m][:].rearrange("p i -> i p")
                        [:, h0 * 16:(h0 + 1) * 16].bitcast(F32R))
                    add_dep_helper(ai.ins, attn_w[m].ins, reason="attn spill RAW")
                amat.append(A)

            # ---------------- stage B: apply + out-proj ----------------
            for w in range(NWIN):
                vt = [bp.tile([128, 512], F32R, tag=f"vt{ch}", name=f"vt{ch}_{w}")
                      for ch in range(2)]
                oT = [bp.tile([128, 512], F32R, tag=f"oT{ch}", name=f"oT{ch}_{w}")
                      for ch in range(2)]
                for ch in range(2):
                    half = (w % 2) * 512
                    ld = nc.sync.dma_start(
                        vt[ch][:],
                        vT_spill[ch][w // 2][:, half:half + 512].bitcast(F32R))
                    add_dep_helper(ld.ins, spill_insts[(ch, w // 2)].ins,
                                   reason="v spill RAW")
                    op_ = ps512.tile([128, 512], F32, tag="w512")
                    nc.tensor.matmul(op_[:], amat[ch][:], vt[ch][:],
                                     start=True, stop=True)
                    nc.scalar.copy(oT[ch][:], op_[:])
                for i in range(4):
                    sc = w * 4 + i
                    yp = ps512.tile([128, C], F32, tag="w512")
                    for ch in range(2):
                        nc.tensor.matmul(yp[:], oT[ch][:, i * 128:(i + 1) * 128],
                                         wo[ch][:], start=(ch == 0), stop=(ch == 1))
                    ysb = bp.tile([128, C], F32, tag="ysb")
                    nc.scalar.copy(ysb[:], yp[:])
                    nc.sync.dma_start(y_out[sc * 128:(sc + 1) * 128, :], ysb[:])

    nc.compile()
    return nc


def _get_nc():
    if "nc" not in _cache:
        _cache["nc"] = build_nc()
    return _cache["nc"]


def kernel(x, w_qkv, w_dw, w_out, temperature, mean_q, var_q, mean_k, var_k):
    x = np.ascontiguousarray(np.asarray(x, np.float32))
    w_qkv = np.ascontiguousarray(np.asarray(w_qkv, np.float32))
    w_dw = np.ascontiguousarray(np.asarray(w_dw, np.float32).reshape(9, C3))
    w_out = np.ascontiguousarray(np.asarray(w_out, np.float32))
    temperature = np.ascontiguousarray(np.asarray(temperature, np.float32).reshape(HEADS))
    stats = [np.ascontiguousarray(np.asarray(t, np.float32).reshape(S))
             for t in (mean_q, var_q, mean_k, var_k)]

    in_maps = []
    for b in range(B):
        in_maps.append({
            "x": np.ascontiguousarray(x[b].reshape(S, C)),
            "w_qkv": w_qkv,
            "w_dw": w_dw,
            "w_out": w_out,
            "temperature": temperature,
            "mean_q": stats[0], "var_q": stats[1],
            "mean_k": stats[2], "var_k": stats[3],
        })
    global _last_in_maps
    _last_in_maps = in_maps
    nc = _get_nc()
    res = run_bass_kernel_spmd(nc, in_maps, list(range(B)))
    out = np.stack([res.results[b]["out"] for b in range(B)])
    return out.reshape(B, H, W, C)
